# revision 1
# baseline (speedup 1.0000x reference)
"""Trainium2 Bass kernel for nn_Att_PD_layer1 (ragged dual-FCNet attention logits
+ ragged pad + masked softmax), data-parallel over 8 NeuronCores.

Contract: kernel(**inputs) takes the FULL unsharded inputs and returns the FULL
[B, 4, maxlen, K] output. Sharding: 2 whole questions per core (balanced
pairing by token*valid-box rows; each question's 4*len segments stay on one
device). Only (token, valid-box) rows go through the GEMMs — masked boxes
cannot affect the output (their logits are zeroed by the mask before the
softmax renormalization), which roughly halves the compute.
"""
import sys
import os

sys.path.insert(0, "/opt/trn_rl_repo")
# this axon env has no NTFF profiling hook; a stray BASS_TRACE=1 would crash
os.environ["BASS_NEVER_TRACE"] = "1"

import numpy as np
import ml_dtypes
from contextlib import ExitStack

import concourse.bass as bass
import concourse.tile as tile
from concourse import bacc, mybir
from concourse.bass_interp import get_hw_module
from concourse import bass_utils

F32 = mybir.dt.float32
BF16 = mybir.dt.bfloat16
AF = mybir.ActivationFunctionType
BF = ml_dtypes.bfloat16

B, G, ML, K = 16, 4, 16, 36
VD, QD, NH = 1024, 1024, 1024
NEG_SLOPE = 0.01

TPC = 112                # max tokens per core
ROWS = 1792              # max packed (token, valid-box) rows per core (14*128)
NCHK = ROWS // 128       # scatter chunks
RCNS = (448, 448, 448, 448)   # rows per chunk; scatter fires on 128-aligned prefixes
NCORES = 8

LAST_RESULT = None       # test harness can inspect results

_CACHE = {}
_TIMING_REPS = None      # when set, wraps the main body in a For_i (timing only)


def _build_program():
    nc = bacc.Bacc("TRN2", target_bir_lowering=False, debug=False,
                   num_devices=NCORES)

    # ---- DRAM I/O (per-core shapes; same program on all 8 cores) ----
    vb = nc.dram_tensor("vb", [ROWS, VD], BF16, kind="ExternalInput")
    qb = nc.dram_tensor("qb", [TPC, QD], BF16, kind="ExternalInput")
    w1v = nc.dram_tensor("w1v", [VD, NH], BF16, kind="ExternalInput")
    w1q = nc.dram_tensor("w1q", [QD, NH], BF16, kind="ExternalInput")
    wg1v = nc.dram_tensor("wg1v", [VD, NH], BF16, kind="ExternalInput")
    wg1q = nc.dram_tensor("wg1q", [QD, NH], BF16, kind="ExternalInput")
    w2 = nc.dram_tensor("w2", [NH, NH], BF16, kind="ExternalInput")
    wg2 = nc.dram_tensor("wg2", [NH, NH], BF16, kind="ExternalInput")
    wlin = nc.dram_tensor("wlin", [128, 8], BF16, kind="ExternalInput")
    b1d = nc.dram_tensor("b1d", [128, 8], F32, kind="ExternalInput")
    b2d = nc.dram_tensor("b2d", [128, 8], F32, kind="ExternalInput")
    bg1d = nc.dram_tensor("bg1d", [128, 8], F32, kind="ExternalInput")
    bg2d = nc.dram_tensor("bg2d", [128, 8], F32, kind="ExternalInput")
    blind = nc.dram_tensor("blind", [1, 1], F32, kind="ExternalInput")
    seld = nc.dram_tensor("seld", [128, ROWS], BF16, kind="ExternalInput")
    maskd = nc.dram_tensor("maskd", [128, K], F32, kind="ExternalInput")
    escatd = nc.dram_tensor("escatd", [128, NCHK, 128], F32, kind="ExternalInput")
    mscatd = nc.dram_tensor("mscatd", [128, NCHK, K], F32, kind="ExternalInput")
    idend = nc.dram_tensor("idend", [128, 128], BF16, kind="ExternalInput")

    outd = nc.dram_tensor("outd", [128, K], F32, kind="ExternalOutput")

    with tile.TileContext(nc) as tc:
        with ExitStack() as ctx:
            wpool = ctx.enter_context(tc.tile_pool(name="weights", bufs=1))
            const = ctx.enter_context(tc.tile_pool(name="const", bufs=1))
            vload = ctx.enter_context(tc.tile_pool(name="vload", bufs=2))
            acts = ctx.enter_context(tc.tile_pool(name="acts", bufs=2))
            small = ctx.enter_context(tc.tile_pool(name="small", bufs=3))
            psum = ctx.enter_context(tc.tile_pool(name="psum", bufs=5, space="PSUM"))
            psq = ctx.enter_context(tc.tile_pool(name="psq", bufs=2, space="PSUM"))
            pspad = ctx.enter_context(tc.tile_pool(name="pspad", bufs=1, space="PSUM"))
            dram = ctx.enter_context(tc.tile_pool(name="dram", bufs=1, space="DRAM"))

            # ---- resident weights: [fin_in_chunk(128), fin_chunk(8), fout(1024)]
            def load_w(dt_, tag):
                t = wpool.tile([128, 8, NH], BF16, tag=tag)
                nc.gpsimd.dma_start(t[:], dt_.ap().rearrange("(kc p) f -> p kc f", p=128))
                return t

            # startup-critical loads first; everything else is emitted later
            # (Tile DMA priority follows emission order) so the first row
            # chunk's matmuls are not starved behind bytes needed later.
            w1v_t = load_w(w1v, "w1v")
            w1q_t = load_w(w1q, "w1q")
            iden_t = const.tile([128, 128], BF16)
            nc.gpsimd.dma_start(iden_t[:], idend.ap())
            q_nat = const.tile([128, QD], BF16)
            nc.vector.memset(q_nat[:], 0.0)
            nc.gpsimd.dma_start(q_nat[:TPC, :], qb.ap())

            wlin_t = const.tile([128, 8], BF16)
            nc.gpsimd.dma_start(wlin_t[:], wlin.ap())
            b1_t = const.tile([128, 8], F32)
            nc.gpsimd.dma_start(b1_t[:], b1d.ap())
            b2_t = const.tile([128, 8], F32)
            nc.gpsimd.dma_start(b2_t[:], b2d.ap())
            bg1_t = const.tile([128, 8], F32)
            nc.gpsimd.dma_start(bg1_t[:], bg1d.ap())
            bg2_t = const.tile([128, 8], F32)
            nc.gpsimd.dma_start(bg2_t[:], bg2d.ap())
            blin_t = const.tile([1, 1], F32)
            nc.gpsimd.dma_start(blin_t[:], blind.ap())
            sel_t = const.tile([128, ROWS], BF16)
            nc.gpsimd.dma_start(sel_t[:], seld.ap())

            # ---- q transpose: qT[p, fc, t] = q[t, fc*128+p]
            qT = const.tile([128, 8, 128], BF16)
            for fc in range(8):
                pst = psq.tile([128, 128], BF16, tag="aux")
                nc.tensor.transpose(pst[:], q_nat[:, fc * 128:(fc + 1) * 128],
                                    iden_t[:])
                nc.vector.tensor_copy(qT[:, fc, :], pst[:])

            # ---- U = q @ Wq  (per branch)  [t(128 part), fout(1024)] bf16
            def compute_u(name, wq):
                ut = const.tile([128, NH], BF16, tag=f"U{name}")
                for nchunk in range(2):
                    ps = psq.tile([128, 512], F32, tag="aux")
                    for kc in range(8):
                        nc.tensor.matmul(
                            ps[:], qT[:, kc, :],
                            wq[:, kc, nchunk * 512:(nchunk + 1) * 512],
                            start=(kc == 0), stop=(kc == 7))
                    nc.vector.tensor_copy(ut[:, nchunk * 512:(nchunk + 1) * 512],
                                          ps[:])
                return ut

            U = {"h": compute_u("h", w1q_t)}
            wg1q_t = load_w(wg1q, "wg1q")
            U["g"] = compute_u("g", wg1q_t)
            wg1v_t = load_w(wg1v, "wg1v")

            # ---- logits row accumulator [1, ROWS] f32
            lrow = const.tile([1, ROWS], F32)
            # padded logits accumulate here across the incremental scatter
            padded_ps = pspad.tile([128, K], F32, tag="padded")

            late = {}

            def late_loads():
                late["w2"] = load_w(w2, "w2")
                late["wg2"] = load_w(wg2, "wg2")

            def tail_loads():
                late["escat"] = const.tile([128, NCHK, 128], F32, tag="escat_t", name="escat_t")
                nc.gpsimd.dma_start(late["escat"][:], escatd.ap())
                late["mscat"] = const.tile([128, NCHK, K], F32, tag="mscat_t", name="mscat_t")
                nc.gpsimd.dma_start(late["mscat"][:], mscatd.ap())
                late["mask"] = const.tile([128, K], F32, tag="mask_t", name="mask_t")
                nc.gpsimd.dma_start(late["mask"][:], maskd.ap())

            def main_body():
                # ---- main loop over row chunks
                r0 = 0
                sc0 = 0          # rows scattered so far
                for rc, rcn in enumerate(RCNS):
                    # vT[p, fc, r] = v[r0+r, fc*128+p]  via HW DMA-transpose
                    vT = vload.tile([128, 8, rcn], BF16)
                    for fc in range(8):
                        nc.sync.dma_start(
                            out=vT[:, fc, :],
                            in_=vb.ap()[r0:r0 + rcn, fc * 128:(fc + 1) * 128],
                            transpose=True)

                    h1T = acts.tile([128, 8, rcn], BF16, tag="h1T")
                    g1T = acts.tile([128, 8, rcn], BF16, tag="g1T")
                    hgT = acts.tile([128, 8, rcn], BF16, tag="hgT")

                    # layer 1: h branch (Lrelu), then g branch (Sigmoid)
                    def l1_branch(name, wv, bt, ot, fn):
                        for mc in range(8):
                            ps = psum.tile([128, rcn], F32, tag="ps")
                            for kc in range(8):
                                nc.tensor.matmul(
                                    ps[:], wv[:, kc, mc * 128:(mc + 1) * 128],
                                    vT[:, kc, :], start=(kc == 0), stop=False)
                            nc.tensor.matmul(
                                ps[:], U[name][:, mc * 128:(mc + 1) * 128],
                                sel_t[:, r0:r0 + rcn], start=False, stop=True)
                            nc.scalar.activation(ot[:, mc, :], ps[:], fn,
                                                 bias=bt[:, mc:mc + 1],
                                                 alpha=NEG_SLOPE)

                    l1_branch("h", w1v_t, b1_t, h1T, AF.Lrelu)
                    l1_branch("g", wg1v_t, bg1_t, g1T, AF.Sigmoid)

                    if rc == 0 and "w2" not in late:
                        # late-needed loads: lower DMA priority than the above
                        late_loads()
                    w2_t, wg2_t = late["w2"], late["wg2"]

                    # layer 2: g branch first (keeps ACT on Sigmoid), then h branch
                    g2T = small.tile([128, 8, rcn], BF16, tag="g2T")
                    for mc in range(8):
                        ps = psum.tile([128, rcn], F32)
                        for kc in range(8):
                            nc.tensor.matmul(
                                ps[:], wg2_t[:, kc, mc * 128:(mc + 1) * 128],
                                g1T[:, kc, :], start=(kc == 0), stop=(kc == 7))
                        nc.scalar.activation(g2T[:, mc, :], ps[:], AF.Sigmoid,
                                             bias=bg2_t[:, mc:mc + 1])
                    for mc in range(8):
                        ps = psum.tile([128, rcn], F32)
                        for kc in range(8):
                            nc.tensor.matmul(
                                ps[:], w2_t[:, kc, mc * 128:(mc + 1) * 128],
                                h1T[:, kc, :], start=(kc == 0), stop=(kc == 7))
                        h2t = small.tile([128, rcn], BF16, tag="h2t")
                        nc.scalar.activation(h2t[:], ps[:], AF.Lrelu,
                                             bias=b2_t[:, mc:mc + 1],
                                             alpha=NEG_SLOPE)
                        nc.vector.tensor_mul(hgT[:, mc, :], h2t[:], g2T[:, mc, :])

                    # final: logits[r] = hg[:, r] . wlin + blin
                    psl = psq.tile([1, rcn], F32, tag="aux")
                    for kc in range(8):
                        nc.tensor.matmul(psl[:], wlin_t[:, kc:kc + 1],
                                         hgT[:, kc, :], start=(kc == 0),
                                         stop=(kc == 7))
                    nc.scalar.activation(lrow[:, r0:r0 + rcn], psl[:], AF.Identity,
                                         bias=blin_t[:, 0:1])

                    if rc == 0 and "escat" not in late:
                        # tail-only constants: lowest useful DMA priority
                        tail_loads()
                    escat_t, mscat_t, mask_t = late["escat"], late["mscat"], late["mask"]

                    # incremental ragged scatter of the finished 128-aligned
                    # lrow prefix: SBUF -> DRAM -> [128, ncc] columns, then 0/1
                    # matmuls accumulate each packed row into its
                    # (padded_row, box) slot of padded_ps.
                    r0 += rcn
                    if r0 % 128 == 0 and r0 > sc0:
                        c0, ncc = sc0 // 128, (r0 - sc0) // 128
                        scr = dram.tile([1, r0 - sc0], F32, tag=f"scr{rc}")
                        nc.gpsimd.dma_start(scr[:], lrow[:, sc0:r0])
                        l2p = small.tile([128, ncc], F32, tag="l2p")
                        nc.gpsimd.dma_start(l2p[:], scr[:].rearrange("x (c p) -> (x p) c", p=128))
                        for cc in range(ncc):
                            c = c0 + cc
                            rhs_c = small.tile([128, K], F32, tag="rhs_c")
                            nc.vector.tensor_scalar_mul(rhs_c[:], mscat_t[:, c, :],
                                                        l2p[:, cc:cc + 1])
                            nc.tensor.matmul(padded_ps[:], escat_t[:, c, :], rhs_c[:],
                                             start=(c == 0), stop=(c == NCHK - 1))
                        sc0 = r0

                # ---- masked softmax tail (f32, exact reference semantics)
                vecm = small.tile([128, K], F32, tag="vecm")
                nc.vector.tensor_mul(vecm[:], padded_ps[:], mask_t[:])
                negmx = small.tile([128, 1], F32, tag="negmx")
                nc.vector.reduce_max(negmx[:], vecm[:], axis=mybir.AxisListType.X,
                                     negate=True)
                e = small.tile([128, K], F32, tag="e")
                nc.scalar.activation(e[:], vecm[:], AF.Exp, bias=negmx[:])
                z = small.tile([128, 1], F32, tag="z")
                nc.vector.reduce_sum(z[:], e[:], axis=mybir.AxisListType.X)
                em = small.tile([128, K], F32, tag="em")
                nc.vector.tensor_mul(em[:], e[:], mask_t[:])
                s2 = small.tile([128, 1], F32, tag="s2")
                nc.vector.reduce_sum(s2[:], em[:], axis=mybir.AxisListType.X)
                den = small.tile([128, 1], F32, tag="den")
                nc.vector.tensor_scalar_mul(den[:], z[:], 1e-13)
                nc.vector.tensor_add(den[:], den[:], s2[:])
                rec = small.tile([128, 1], F32, tag="rec")
                nc.vector.reciprocal(rec[:], den[:])
                outt = small.tile([128, K], F32, tag="outt")
                nc.vector.tensor_scalar_mul(outt[:], em[:], rec[:])
                nc.sync.dma_start(outd.ap(), outt[:])

            if _TIMING_REPS:
                late_loads()
                tail_loads()
                with tc.For_i(0, _TIMING_REPS, 1):
                    main_body()
            else:
                main_body()

    nc.compile()
    nc.m = get_hw_module(nc.m)
    return nc


def _pair_questions(weight):
    """Greedy balanced pairing: sort desc, pair largest with smallest."""
    order = np.argsort(-np.asarray(weight), kind="stable")
    pairs = []
    lo, hi = 0, len(order) - 1
    while lo < hi:
        pairs.append((int(order[lo]), int(order[hi])))
        lo += 1
        hi -= 1
    return pairs


def kernel(v, q, box_mask, tags_attention, W1, b1, W2, b2, Wg1, bg1, Wg2, bg2,
           w_lin, b_lin):
    global LAST_RESULT
    v = np.asarray(v, dtype=np.float32)
    q = np.asarray(q, dtype=np.float32)
    box_mask = np.asarray(box_mask, dtype=np.float32)
    tags_attention = np.asarray(tags_attention)

    lengths = tags_attention.sum(-1).astype(np.int64)          # [B, G]
    qlen = lengths.sum(-1)                                     # [B]
    qstart = np.concatenate([[0], np.cumsum(qlen)[:-1]])
    valid_ks = [np.where(box_mask[b] > 0)[0] for b in range(B)]
    nval = np.array([len(vk) for vk in valid_ks])
    pairs = _pair_questions(qlen * nval)
    assert len(pairs) == NCORES
    assert max(qlen[a] + qlen[b] for a, b in pairs) <= TPC
    assert max(qlen[a] * nval[a] + qlen[b] * nval[b] for a, b in pairs) <= ROWS

    # shared (per-core identical) tensors
    wb = {
        "w1v": np.ascontiguousarray(W1[:VD]).astype(BF),
        "w1q": np.ascontiguousarray(W1[VD:]).astype(BF),
        "wg1v": np.ascontiguousarray(Wg1[:VD]).astype(BF),
        "wg1q": np.ascontiguousarray(Wg1[VD:]).astype(BF),
        "w2": np.asarray(W2).astype(BF),
        "wg2": np.asarray(Wg2).astype(BF),
        "wlin": np.asarray(w_lin).reshape(8, 128).T.copy().astype(BF),
        "b1d": np.asarray(b1).astype(np.float32).reshape(8, 128).T.copy(),
        "b2d": np.asarray(b2).astype(np.float32).reshape(8, 128).T.copy(),
        "bg1d": np.asarray(bg1).astype(np.float32).reshape(8, 128).T.copy(),
        "bg2d": np.asarray(bg2).astype(np.float32).reshape(8, 128).T.copy(),
        "blind": np.asarray(b_lin).astype(np.float32).reshape(1, 1),
        "idend": np.eye(128, dtype=np.float32).astype(BF),
    }

    in_maps = []
    for c in range(NCORES):
        b0, b1q = pairs[c]
        ntok0, ntok1 = int(qlen[b0]), int(qlen[b1q])
        ntok = ntok0 + ntok1
        qs = np.zeros((TPC, QD), dtype=np.float32)
        qs[:ntok0] = q[qstart[b0]:qstart[b0] + ntok0]
        qs[ntok0:ntok] = q[qstart[b1q]:qstart[b1q] + ntok1]

        # packed (token, valid-box) rows
        vs = np.zeros((ROWS, VD), dtype=np.float32)
        sel = np.zeros((128, ROWS), dtype=np.float32)
        escat = np.zeros((128, NCHK, 128), dtype=np.float32)
        mscat = np.zeros((128, NCHK, K), dtype=np.float32)
        mask128 = np.zeros((128, K), dtype=np.float32)
        r = 0
        for lq, bq in enumerate((b0, b1q)):
            vk = valid_ks[bq]
            ntk = int(qlen[bq])
            tl0 = 0 if lq == 0 else ntok0           # local token base
            vrows = v[qstart[bq]:qstart[bq] + ntk][:, vk, :]  # [ntk, nv, VD]
            nv = len(vk)
            vs[r:r + ntk * nv] = vrows.reshape(ntk * nv, VD)
            # per-row metadata
            t_loc = tl0 + np.repeat(np.arange(ntk), nv)
            kbox = np.tile(vk, ntk)
            rows = np.arange(r, r + ntk * nv)
            sel[t_loc, rows] = 1.0
            # padded row index p for each packed row: (lq*4+g)*16 + pos
            loc = np.concatenate([[0], np.cumsum(lengths[bq])[:-1]])
            # map token local-in-question -> (g, pos)
            gg = np.concatenate([np.full(int(lengths[bq, g]), g) for g in range(G)])
            pp = np.concatenate([np.arange(int(lengths[bq, g])) for g in range(G)])
            p_of_tok = (lq * G + gg) * ML + pp      # [ntk]
            p_rows = np.repeat(p_of_tok, nv)        # [ntk*nv]
            escat[rows % 128, rows // 128, p_rows] = 1.0
            mscat[rows % 128, rows // 128, kbox] = 1.0
            mask128[lq * G * ML:(lq + 1) * G * ML] = box_mask[bq][None, :]
            r += ntk * nv

        m = dict(wb)
        m["vb"] = vs.astype(BF)
        m["qb"] = qs.astype(BF)
        m["seld"] = sel.astype(BF)
        m["maskd"] = mask128
        m["escatd"] = escat
        m["mscatd"] = mscat
        in_maps.append(m)

    if "nc" not in _CACHE:
        _CACHE["nc"] = _build_program()
    nc = _CACHE["nc"]

    LAST_RESULT = bass_utils.run_bass_kernel_spmd(
        nc, in_maps, core_ids=list(range(NCORES)))

    out = np.zeros((B, G, ML, K), dtype=np.float32)
    for c in range(NCORES):
        b0, b1q = pairs[c]
        r = LAST_RESULT.results[c]["outd"]
        out[b0] = r[:G * ML].reshape(G, ML, K)
        out[b1q] = r[G * ML:].reshape(G, ML, K)
    return out



# revision 7
# speedup vs baseline: 2.1709x; 2.1709x over previous
"""Trainium2 Bass kernel for nn_Att_PD_layer1 (ragged dual-FCNet attention logits
+ ragged pad + masked softmax), data-parallel over 8 NeuronCores.

Contract: kernel(**inputs) takes the FULL unsharded inputs and returns the FULL
[B, 4, maxlen, K] output. Sharding: 2 whole questions per core (balanced
pairing by token*valid-box rows). Only (token, valid-box) rows go through the
GEMMs.

v2: layer-1 v-GEMMs run as fp8e4 DoubleRow matmuls (2 k-tiles of 128 per
instruction, ~1.76x bf16 throughput on HW). Weights are host-quantized at x64
scale; the 1/64 rescale folds into the layer-2 h-weights (leaky-relu is
positively homogeneous) and into the sigmoid drain's scale. Leaky-relu drains
run on the Pool engine (scalar_tensor_tensor max(0.01x, x)); sigmoid drains
stay on ACT with a single table. v arrives host-pre-transposed fp8 in
chunk-major layout (no DMA transposes). The ragged scatter transposes the
logit row via tiny PE matmuls instead of a DRAM round trip.
"""
import sys
import os

sys.path.insert(0, "/opt/trn_rl_repo")
# this axon env has no NTFF profiling hook; a stray BASS_TRACE=1 would crash
os.environ["BASS_NEVER_TRACE"] = "1"

import numpy as np
import ml_dtypes
from contextlib import ExitStack

import concourse.bass as bass
import concourse.tile as tile
from concourse import bacc, mybir
from concourse.bass_interp import get_hw_module
from concourse import bass_utils

F32 = mybir.dt.float32
BF16 = mybir.dt.bfloat16
FP8 = mybir.dt.float8e4
AF = mybir.ActivationFunctionType
ALU = mybir.AluOpType
DR = mybir.MatmulPerfMode.DoubleRow
BF = ml_dtypes.bfloat16
E4 = ml_dtypes.float8_e4m3fn

B, G, ML, K = 16, 4, 16, 36
VD, QD, NH = 1024, 1024, 1024
NEG_SLOPE = 0.01
SW = 64.0            # fp8 weight scale

TPC = 112                 # max tokens per core
ROWS = 1792               # max packed (token, valid-box) rows per core
RCNS = (512, 512, 512, 256)   # rows per chunk (128-multiples)
NCHK = ROWS // 128        # 14 scatter column-chunks
NCORES = 8

LAST_RESULT = None
_CACHE = {}
_TIMING_REPS = None       # when set, wraps the main body in a For_i (timing only)


def _build_program():
    nc = bacc.Bacc("TRN2", target_bir_lowering=False, debug=False,
                   num_devices=NCORES)

    # ---- DRAM I/O (per-core shapes; same program on all 8 cores) ----
    # v pre-transposed+quantized on host: chunk-major [128, sum(8*rcn)]
    vbT = nc.dram_tensor("vbT", [128, 8 * ROWS], FP8, kind="ExternalInput")
    qb = nc.dram_tensor("qb", [TPC, QD], BF16, kind="ExternalInput")
    w1q = nc.dram_tensor("w1q", [128, 8, NH], BF16, kind="ExternalInput")
    wg1q = nc.dram_tensor("wg1q", [128, 8, NH], BF16, kind="ExternalInput")
    w1v8 = nc.dram_tensor("w1v8", [128, 8, NH], FP8, kind="ExternalInput")
    wg1v8 = nc.dram_tensor("wg1v8", [128, 8, NH], FP8, kind="ExternalInput")
    w2s = nc.dram_tensor("w2s", [128, 8, NH], BF16, kind="ExternalInput")   # W2/64
    wg2 = nc.dram_tensor("wg2", [128, 8, NH], BF16, kind="ExternalInput")
    wlin = nc.dram_tensor("wlin", [128, 8], BF16, kind="ExternalInput")
    b1x64 = nc.dram_tensor("b1x64", [128, 8], F32, kind="ExternalInput")
    bg1d = nc.dram_tensor("bg1d", [128, 8], F32, kind="ExternalInput")
    b2d = nc.dram_tensor("b2d", [128, 8], F32, kind="ExternalInput")
    bg2d = nc.dram_tensor("bg2d", [128, 8], F32, kind="ExternalInput")
    blind = nc.dram_tensor("blind", [1, 1], F32, kind="ExternalInput")
    seld = nc.dram_tensor("seld", [128, ROWS], BF16, kind="ExternalInput")
    maskd = nc.dram_tensor("maskd", [128, K], F32, kind="ExternalInput")
    escatd = nc.dram_tensor("escatd", [128, NCHK, 128], F32, kind="ExternalInput")
    mscatd = nc.dram_tensor("mscatd", [128, NCHK, K], F32, kind="ExternalInput")
    idend = nc.dram_tensor("idend", [128, 128], BF16, kind="ExternalInput")
    ones11d = nc.dram_tensor("ones11d", [1, 1], F32, kind="ExternalInput")

    outd = nc.dram_tensor("outd", [128, K], F32, kind="ExternalOutput")

    with tile.TileContext(nc) as tc:
        with ExitStack() as ctx:
            wpool = ctx.enter_context(tc.tile_pool(name="weights", bufs=1))
            const = ctx.enter_context(tc.tile_pool(name="const", bufs=1))
            vload = ctx.enter_context(tc.tile_pool(name="vload", bufs=2))
            acts = ctx.enter_context(tc.tile_pool(name="acts", bufs=2))
            small = ctx.enter_context(tc.tile_pool(name="small", bufs=3))
            psum = ctx.enter_context(tc.tile_pool(name="psum", bufs=4, space="PSUM"))
            psq = ctx.enter_context(tc.tile_pool(name="psq", bufs=2, space="PSUM"))
            pspad = ctx.enter_context(tc.tile_pool(name="pspad", bufs=1, space="PSUM"))

            def load_w(dt_, tag, dtype):
                t = wpool.tile([128, 8, NH], dtype, tag=tag)
                nc.gpsimd.dma_start(t[:], dt_.ap())
                return t

            # startup-critical loads first (DMA priority follows emission
            # order): U-path weights, then the fp8 layer-1 weights.
            q_nat = const.tile([128, QD], BF16)
            nc.vector.memset(q_nat[:], 0.0)
            nc.gpsimd.dma_start(q_nat[:TPC, :], qb.ap())
            iden_t = const.tile([128, 128], BF16)
            nc.gpsimd.dma_start(iden_t[:], idend.ap())
            w1q_t = load_w(w1q, "w1q", BF16)
            w1v_t = load_w(w1v8, "w1v8", FP8)
            wg1q_t = load_w(wg1q, "wg1q", BF16)
            wg1v_t = load_w(wg1v8, "wg1v8", FP8)

            wlin_t = const.tile([128, 8], BF16)
            nc.gpsimd.dma_start(wlin_t[:], wlin.ap())
            b1x64_t = const.tile([128, 8], F32)
            nc.gpsimd.dma_start(b1x64_t[:], b1x64.ap())
            bg1_t = const.tile([128, 8], F32)
            nc.gpsimd.dma_start(bg1_t[:], bg1d.ap())
            b2_t = const.tile([128, 8], F32)
            nc.gpsimd.dma_start(b2_t[:], b2d.ap())
            bg2_t = const.tile([128, 8], F32)
            nc.gpsimd.dma_start(bg2_t[:], bg2d.ap())
            blin_t = const.tile([1, 1], F32)
            nc.gpsimd.dma_start(blin_t[:], blind.ap())
            ones11_t = const.tile([1, 1], F32)
            nc.gpsimd.dma_start(ones11_t[:], ones11d.ap())
            sel_t = const.tile([128, ROWS], BF16)
            nc.gpsimd.dma_start(sel_t[:], seld.ap())

            # ---- q transpose: qT[p, fc, t] = q[t, fc*128+p]
            qT = const.tile([128, 8, 128], BF16)
            for fc in range(8):
                pst = psq.tile([128, 512], BF16, tag="aux")
                nc.tensor.transpose(pst[:, :128], q_nat[:, fc * 128:(fc + 1) * 128],
                                    iden_t[:])
                nc.vector.tensor_copy(qT[:, fc, :], pst[:, :128])

            # ---- U = (q @ Wq + b) * 64  [t(128 part), fout(1024)] bf16
            def compute_u(name, wq, bias_t):
                ut = const.tile([128, NH], BF16, tag=f"U{name}")
                for nchunk in range(2):
                    ps = psq.tile([128, 512], F32, tag="aux")
                    for kc in range(8):
                        nc.tensor.matmul(
                            ps[:], qT[:, kc, :],
                            wq[:, kc, nchunk * 512:(nchunk + 1) * 512],
                            start=(kc == 0), stop=(kc == 7))
                    if bias_t is None:
                        nc.scalar.activation(
                            ut[:, nchunk * 512:(nchunk + 1) * 512], ps[:],
                            AF.Identity, scale=SW)
                    else:
                        for j in range(4):
                            c = nchunk * 4 + j
                            nc.scalar.activation(
                                ut[:, c * 128:(c + 1) * 128],
                                ps[:, j * 128:(j + 1) * 128],
                                AF.Identity, scale=SW, bias=bias_t[:, c:c + 1])
                return ut

            U = {"h": compute_u("h", w1q_t, b1x64_t),
                 "g": compute_u("g", wg1q_t, None)}

            # logits row accumulator [1, ROWS] f32 (partition 0)
            lrow = const.tile([1, ROWS], F32)
            padded_ps = pspad.tile([128, K], F32, tag="padded")

            late = {}

            def late_loads():
                late["wg2"] = load_w(wg2, "wg2", BF16)
                late["w2"] = load_w(w2s, "w2s", BF16)

            def tail_loads():
                late["escat"] = const.tile([128, NCHK, 128], F32, tag="escat_t",
                                           name="escat_t")
                nc.gpsimd.dma_start(late["escat"][:], escatd.ap())
                late["mscat"] = const.tile([128, NCHK, K], F32, tag="mscat_t",
                                           name="mscat_t")
                nc.gpsimd.dma_start(late["mscat"][:], mscatd.ap())
                late["mask"] = const.tile([128, K], F32, tag="mask_t",
                                          name="mask_t")
                nc.gpsimd.dma_start(late["mask"][:], maskd.ap())

            def main_body():
                r0 = 0
                off = 0          # element offset into vbT (chunk-major)
                for rc, rcn in enumerate(RCNS):
                    vT = vload.tile([128, 8, 512], FP8)
                    nc.sync.dma_start(
                        vT[:, :, :rcn],
                        vbT.ap()[:, off:off + 8 * rcn].rearrange(
                            "p (kc r) -> p kc r", kc=8))

                    h1T = acts.tile([128, 8, 512], BF16, tag="h1T")
                    g1T = acts.tile([128, 8, 512], BF16, tag="g1T")
                    hgT = acts.tile([128, 8, 512], BF16, tag="hgT")

                    # layer 1: 4 fp8 DoubleRow matmuls + bf16 sel/U matmul
                    def l1_mm(wv, uname, mc, ps):
                        for kp in range(4):
                            nc.tensor.matmul(
                                ps[:, :rcn],
                                wv[:, 2 * kp:2 * kp + 2, mc * 128:(mc + 1) * 128],
                                vT[:, 2 * kp:2 * kp + 2, :rcn],
                                start=(kp == 0), stop=False, perf_mode=DR)
                        nc.tensor.matmul(
                            ps[:, :rcn], U[uname][:, mc * 128:(mc + 1) * 128],
                            sel_t[:, r0:r0 + rcn], start=False, stop=True)

                    # h branch: drain = parametric-relu on ACT (64x scale
                    # stays -- prelu is positively homogeneous; b1 already
                    # folded into U-h). Prelu shares the sigmoid act table.
                    for mc in range(8):
                        ps = psum.tile([128, 512], F32, tag="ps")
                        l1_mm(w1v_t, "h", mc, ps)
                        nc.scalar.activation(h1T[:, mc, :rcn], ps[:, :rcn],
                                             AF.Prelu, alpha=NEG_SLOPE)
                    # g branch: drain = sigmoid on ACT (scale 1/64, bias bg1)
                    for mc in range(8):
                        ps = psum.tile([128, 512], F32, tag="ps")
                        l1_mm(wg1v_t, "g", mc, ps)
                        nc.scalar.activation(g1T[:, mc, :rcn], ps[:, :rcn],
                                             AF.Sigmoid, scale=1.0 / SW,
                                             bias=bg1_t[:, mc:mc + 1])

                    if rc == 0 and "w2" not in late:
                        late_loads()
                    w2_t, wg2_t = late["w2"], late["wg2"]

                    # layer 2 g: bf16 matmuls, sigmoid drain on ACT
                    g2T = small.tile([128, 8, 512], BF16, tag="g2T")
                    for mc in range(8):
                        ps = psum.tile([128, 512], F32, tag="ps")
                        for kc in range(8):
                            nc.tensor.matmul(
                                ps[:, :rcn], wg2_t[:, kc, mc * 128:(mc + 1) * 128],
                                g1T[:, kc, :rcn], start=(kc == 0), stop=(kc == 7))
                        nc.scalar.activation(g2T[:, mc, :rcn], ps[:, :rcn],
                                             AF.Sigmoid, bias=bg2_t[:, mc:mc + 1])
                    # layer 2 h: bf16 matmuls (W2/64 vs 64-scaled h1), then
                    # parametric-relu drain on ACT (Prelu shares the sigmoid
                    # act table; bias=b2), then h2*g2 on DVE
                    for mc in range(8):
                        ps = psum.tile([128, 512], F32, tag="ps")
                        for kc in range(8):
                            nc.tensor.matmul(
                                ps[:, :rcn], w2_t[:, kc, mc * 128:(mc + 1) * 128],
                                h1T[:, kc, :rcn], start=(kc == 0), stop=(kc == 7))
                        h2t = small.tile([128, 512], BF16, tag="h2t")
                        nc.scalar.activation(h2t[:, :rcn], ps[:, :rcn], AF.Prelu,
                                             bias=b2_t[:, mc:mc + 1],
                                             alpha=NEG_SLOPE)
                        nc.vector.tensor_mul(hgT[:, mc, :rcn], h2t[:, :rcn],
                                             g2T[:, mc, :rcn])

                    # final: logits[r] = hg[:, r] . wlin + blin
                    psl = psq.tile([1, 512], F32, tag="aux")
                    for kc in range(8):
                        nc.tensor.matmul(psl[:, :rcn], wlin_t[:, kc:kc + 1],
                                         hgT[:, kc, :rcn], start=(kc == 0),
                                         stop=(kc == 7))
                    nc.vector.tensor_scalar_add(lrow[:, r0:r0 + rcn],
                                                psl[:, :rcn], blin_t[:, 0:1])

                    if rc == 0 and "escat" not in late:
                        tail_loads()
                    escat_t, mscat_t, mask_t = (late["escat"], late["mscat"],
                                                late["mask"])

                    # incremental ragged scatter of finished 128-blocks:
                    # transpose lrow pieces to partitions via 1-wide matmuls,
                    # then 0/1 matmuls accumulate into padded_ps.
                    c0, ncc = r0 // 128, rcn // 128
                    r0 += rcn
                    off += 8 * rcn
                    scps = pspad.tile([128, 4], F32, tag="scps")
                    for cc in range(ncc):
                        c = c0 + cc
                        nc.tensor.matmul(
                            scps[:, cc:cc + 1],
                            lrow[0:1, c * 128:(c + 1) * 128], ones11_t[:],
                            start=True, stop=True, skip_group_check=True)
                        rhs_c = small.tile([128, K], F32, tag="rhs_c")
                        nc.vector.tensor_scalar_mul(rhs_c[:], mscat_t[:, c, :],
                                                    scps[:, cc:cc + 1])
                        nc.tensor.matmul(padded_ps[:], escat_t[:, c, :], rhs_c[:],
                                         start=(c == 0), stop=(c == NCHK - 1),
                                         skip_group_check=True)

                # ---- masked softmax tail (f32, exact reference semantics)
                vecm = small.tile([128, K], F32, tag="vecm")
                nc.vector.tensor_mul(vecm[:], padded_ps[:], mask_t[:])
                negmx = small.tile([128, 1], F32, tag="negmx")
                nc.vector.reduce_max(negmx[:], vecm[:], axis=mybir.AxisListType.X,
                                     negate=True)
                e = small.tile([128, K], F32, tag="e")
                nc.scalar.activation(e[:], vecm[:], AF.Exp, bias=negmx[:])
                z = small.tile([128, 1], F32, tag="z")
                nc.vector.reduce_sum(z[:], e[:], axis=mybir.AxisListType.X)
                em = small.tile([128, K], F32, tag="em")
                nc.vector.tensor_mul(em[:], e[:], mask_t[:])
                s2 = small.tile([128, 1], F32, tag="s2")
                nc.vector.reduce_sum(s2[:], em[:], axis=mybir.AxisListType.X)
                den = small.tile([128, 1], F32, tag="den")
                nc.vector.tensor_scalar_mul(den[:], z[:], 1e-13)
                nc.vector.tensor_add(den[:], den[:], s2[:])
                rec = small.tile([128, 1], F32, tag="rec")
                nc.vector.reciprocal(rec[:], den[:])
                outt = small.tile([128, K], F32, tag="outt")
                nc.vector.tensor_scalar_mul(outt[:], em[:], rec[:])
                nc.sync.dma_start(outd.ap(), outt[:])

            if _TIMING_REPS:
                late_loads()
                tail_loads()
                with tc.For_i(0, _TIMING_REPS, 1):
                    main_body()
            else:
                main_body()

    nc.compile()
    nc.m = get_hw_module(nc.m)
    return nc


def _pair_questions(weight):
    """Greedy balanced pairing: sort desc, pair largest with smallest."""
    order = np.argsort(-np.asarray(weight), kind="stable")
    pairs = []
    lo, hi = 0, len(order) - 1
    while lo < hi:
        pairs.append((int(order[lo]), int(order[hi])))
        lo += 1
        hi -= 1
    return pairs


def _rearrange_w(w):
    """[1024, NH] -> [128, 8, NH] with fin = kc*128 + p."""
    return np.ascontiguousarray(w.reshape(8, 128, NH).transpose(1, 0, 2))


def kernel(v, q, box_mask, tags_attention, W1, b1, W2, b2, Wg1, bg1, Wg2, bg2,
           w_lin, b_lin):
    global LAST_RESULT
    v = np.asarray(v, dtype=np.float32)
    q = np.asarray(q, dtype=np.float32)
    box_mask = np.asarray(box_mask, dtype=np.float32)
    tags_attention = np.asarray(tags_attention)

    lengths = tags_attention.sum(-1).astype(np.int64)          # [B, G]
    qlen = lengths.sum(-1)                                     # [B]
    qstart = np.concatenate([[0], np.cumsum(qlen)[:-1]])
    valid_ks = [np.where(box_mask[b] > 0)[0] for b in range(B)]
    nval = np.array([len(vk) for vk in valid_ks])
    pairs = _pair_questions(qlen * nval)
    assert len(pairs) == NCORES
    assert max(qlen[a] + qlen[b] for a, b in pairs) <= TPC
    assert max(qlen[a] * nval[a] + qlen[b] * nval[b] for a, b in pairs) <= ROWS

    W1 = np.asarray(W1, np.float32)
    Wg1 = np.asarray(Wg1, np.float32)
    W2 = np.asarray(W2, np.float32)
    Wg2 = np.asarray(Wg2, np.float32)

    # shared (per-core identical) tensors
    wb = {
        "w1q": _rearrange_w(W1[VD:]).astype(BF),
        "wg1q": _rearrange_w(Wg1[VD:]).astype(BF),
        "w1v8": _rearrange_w(W1[:VD] * SW).astype(E4),
        "wg1v8": _rearrange_w(Wg1[:VD] * SW).astype(E4),
        "w2s": _rearrange_w(W2 / SW).astype(BF),
        "wg2": _rearrange_w(Wg2).astype(BF),
        "wlin": np.asarray(w_lin).reshape(8, 128).T.copy().astype(BF),
        "b1x64": (np.asarray(b1, np.float32) * SW).reshape(8, 128).T.copy(),
        "bg1d": np.asarray(bg1, np.float32).reshape(8, 128).T.copy(),
        "b2d": np.asarray(b2, np.float32).reshape(8, 128).T.copy(),
        "bg2d": np.asarray(bg2, np.float32).reshape(8, 128).T.copy(),
        "blind": np.asarray(b_lin, np.float32).reshape(1, 1),
        "idend": np.eye(128, dtype=np.float32).astype(BF),
        "ones11d": np.ones((1, 1), np.float32),
    }

    in_maps = []
    for c in range(NCORES):
        b0, b1q = pairs[c]
        ntok0, ntok1 = int(qlen[b0]), int(qlen[b1q])
        ntok = ntok0 + ntok1
        qs = np.zeros((TPC, QD), dtype=np.float32)
        qs[:ntok0] = q[qstart[b0]:qstart[b0] + ntok0]
        qs[ntok0:ntok] = q[qstart[b1q]:qstart[b1q] + ntok1]

        # packed (token, valid-box) rows
        vs = np.zeros((ROWS, VD), dtype=np.float32)
        sel = np.zeros((128, ROWS), dtype=np.float32)
        escat = np.zeros((128, NCHK, 128), dtype=np.float32)
        mscat = np.zeros((128, NCHK, K), dtype=np.float32)
        mask128 = np.zeros((128, K), dtype=np.float32)
        r = 0
        for lq, bq in enumerate((b0, b1q)):
            vk = valid_ks[bq]
            ntk = int(qlen[bq])
            tl0 = 0 if lq == 0 else ntok0           # local token base
            vrows = v[qstart[bq]:qstart[bq] + ntk][:, vk, :]  # [ntk, nv, VD]
            nv = len(vk)
            vs[r:r + ntk * nv] = vrows.reshape(ntk * nv, VD)
            t_loc = tl0 + np.repeat(np.arange(ntk), nv)
            kbox = np.tile(vk, ntk)
            rows = np.arange(r, r + ntk * nv)
            sel[t_loc, rows] = 1.0
            gg = np.concatenate([np.full(int(lengths[bq, g]), g) for g in range(G)])
            pp = np.concatenate([np.arange(int(lengths[bq, g])) for g in range(G)])
            p_of_tok = (lq * G + gg) * ML + pp      # [ntk]
            p_rows = np.repeat(p_of_tok, nv)        # [ntk*nv]
            escat[rows % 128, rows // 128, p_rows] = 1.0
            mscat[rows % 128, rows // 128, kbox] = 1.0
            mask128[lq * G * ML:(lq + 1) * G * ML] = box_mask[bq][None, :]
            r += ntk * nv

        # fp8 quantize + chunk-major transpose: [128, 8*rcn] per chunk
        vq8 = vs.astype(E4)
        pieces = []
        r0 = 0
        for rcn in RCNS:
            blk = vq8[r0:r0 + rcn].reshape(rcn, 8, 128).transpose(2, 1, 0)
            pieces.append(np.ascontiguousarray(blk).reshape(128, 8 * rcn))
            r0 += rcn
        vbT = np.concatenate(pieces, axis=1)

        m = dict(wb)
        m["vbT"] = vbT
        m["qb"] = qs.astype(BF)
        m["seld"] = sel.astype(BF)
        m["maskd"] = mask128
        m["escatd"] = escat
        m["mscatd"] = mscat
        in_maps.append(m)

    if "nc" not in _CACHE:
        _CACHE["nc"] = _build_program()
    nc = _CACHE["nc"]

    LAST_RESULT = bass_utils.run_bass_kernel_spmd(
        nc, in_maps, core_ids=list(range(NCORES)))

    out = np.zeros((B, G, ML, K), dtype=np.float32)
    for c in range(NCORES):
        b0, b1q = pairs[c]
        r = LAST_RESULT.results[c]["outd"]
        out[b0] = r[:G * ML].reshape(G, ML, K)
        out[b1q] = r[G * ML:].reshape(G, ML, K)
    return out
